# revision 1
# baseline (speedup 1.0000x reference)
"""CPFGNN Trainium2 kernel: 8-core SPMD Bass implementation.

Math (exact simplifications of the reference):
  - lam = 2.0 always (w_off <= 0), so diag = 0 and prop(t) is a pure
    edge scatter-add: prop(t) = -D^-1/2 A^T D^-1/2 t, with A the
    (multi-)adjacency count matrix excluding self-loops and deg = out-degree.
  - The 11 CTC @ e_k matvecs batch into one CTC @ E (N x 11) pass.

Mapping:
  - Nodes sharded 8 ways (1250 rows each): feature/CTC/A/output row-sharded.
  - prop is computed DENSELY on the TensorEngine: A is shipped as an exact
    fp8(e4m3) count matrix (values 0,1,2.. are exact); the moving operand
    streams A column-blocks; the stationary operand is u = D^-1/2 t split
    into fp8 hi+lo columns (M=20), recovering ~bf16 accuracy.
  - Per hop: 25 KB fp8 AllGather of each core's u-block.
  - CTC is shipped pre-transposed in bf16 and streamed once as the moving
    operand against the stationary E matrix (M=11).
"""
import os
import sys

sys.path.insert(0, "/opt/trn_rl_repo")

import numpy as np
import ml_dtypes
from contextlib import ExitStack

N = 10000
E_EDGES = 320000
F_IN = 500
HID = 64
C = 10
RANK = 3
K = 10
NC = 8
NSH = N // NC              # 1250 nodes per core
JT = (N + 127) // 128      # 79 src tiles (last partial: 16)
JLAST = N - 128 * (JT - 1)  # 16
LT = (NSH + 127) // 128    # 10 local node tiles (last partial: 98)
LLAST = NSH - 128 * (LT - 1)  # 98
STRIPS = [(0, 512), (512, 512), (1024, NSH - 1024)]
# per-core row blocks: each core's 1250 nodes = 9 full 128-tiles + one 98-tile
GJT = NC * LT               # 80 global j-tiles in per-core-tiled order
UW = 128                    # padded fp8 u row (hi 0:10, mid 32:42, lo 64:74)
EW = 32                     # padded bf16 e row (11 used)

NP_FP8 = ml_dtypes.float8_e4m3
NP_BF16 = ml_dtypes.bfloat16

_CACHE = {}


def _build_program():
    import concourse.bass as bass
    import concourse.tile as tile
    from concourse import bacc, mybir
    from concourse.masks import make_identity

    dt = mybir.dt
    FP8 = dt.float8e4
    BF16 = dt.bfloat16
    FP16 = dt.float16
    F32 = dt.float32
    AF = mybir.ActivationFunctionType
    ALU = mybir.AluOpType

    nc = bacc.Bacc("TRN2", target_bir_lowering=False, debug=False, num_devices=NC)

    # ---------------- DRAM I/O ----------------
    a_dram = nc.dram_tensor("a8", [N, NSH], FP8, kind="ExternalInput")
    featT_dram = nc.dram_tensor("featT", [F_IN, NSH], F32, kind="ExternalInput")
    ctct_dram = nc.dram_tensor("ctct", [N, NSH], FP16, kind="ExternalInput")
    w1_dram = nc.dram_tensor("w1", [F_IN, HID], F32, kind="ExternalInput")
    b1_dram = nc.dram_tensor("b1", [HID, 1], F32, kind="ExternalInput")
    w2_dram = nc.dram_tensor("w2", [HID, C], F32, kind="ExternalInput")
    b2_dram = nc.dram_tensor("b2", [C, 1], F32, kind="ExternalInput")
    wp_dram = nc.dram_tensor("wp", [C, (K + 1) * RANK], FP16, kind="ExternalInput")
    bp_dram = nc.dram_tensor("bp", [RANK, K + 1], F32, kind="ExternalInput")
    gam_dram = nc.dram_tensor("gam", [RANK, K + 1], FP16, kind="ExternalInput")
    # rows: 0 = dinv_loc, 1 = -dinv_loc, 2 = -2*dinv_loc   (this core's range)
    dinv_dram = nc.dram_tensor("dinvs", [3, NSH], F32, kind="ExternalInput")
    sel3_dram = nc.dram_tensor("sel3", [3, 30], F32, kind="ExternalInput")
    sel11_dram = nc.dram_tensor("sel11", [K + 1, (K + 1) * C], F32, kind="ExternalInput")
    out_dram = nc.dram_tensor("out", [NSH, C], F32, kind="ExternalOutput")
    DEBUG = bool(os.environ.get("GNN_DEBUG"))
    if DEBUG:
        dump_tx = nc.dram_tensor("dump_tx", [K + 1, C, NSH], F32, kind="ExternalOutput")
        dump_e = nc.dram_tensor("dump_e", [K + 1, NSH], F32, kind="ExternalOutput")
        dump_eta = nc.dram_tensor("dump_eta", [K + 1, NSH], F32, kind="ExternalOutput")
        dump_hid = nc.dram_tensor("dump_hid", [C, NSH], F32, kind="ExternalOutput")
        dump_ustat = nc.dram_tensor("dump_ustat", [128, GJT, UW], F32, kind="ExternalOutput")
        dump_prop = nc.dram_tensor("dump_prop", [C, NSH], F32, kind="ExternalOutput")

    ag_u_in = nc.dram_tensor("ag_u_in", [128, LT, UW], FP8)
    ag_u_out = nc.dram_tensor("ag_u_out", [NC, 128, LT, UW], FP8, addr_space="Shared")
    ag_e_in = nc.dram_tensor("ag_e_in", [128, LT, EW], FP16)
    ag_e_out = nc.dram_tensor("ag_e_out", [NC, 128, LT, EW], FP16, addr_space="Shared")

    RG = [list(range(NC))]


    with ExitStack() as ctx:
        tc = ctx.enter_context(tile.TileContext(nc))
        const = ctx.enter_context(tc.tile_pool(name="const", bufs=1))
        big = ctx.enter_context(tc.tile_pool(name="big", bufs=1))     # [C,NSH]-ish f32 temps
        small = ctx.enter_context(tc.tile_pool(name="small", bufs=3))  # small temps
        stream = ctx.enter_context(tc.tile_pool(name="stream", bufs=3))

        # ------------- resident constants -------------
        w1 = const.tile([128, 4, HID], F32, tag="w1")
        nc.sync.dma_start(
            w1[:, 0:3, :], w1_dram[0:384, :].rearrange("(t p) c -> p t c", p=128)
        )
        nc.sync.dma_start(w1[0:F_IN - 384, 3, :], w1_dram[384:F_IN, :])
        b1 = const.tile([HID, 1], F32, tag="b1")
        nc.sync.dma_start(b1[:], b1_dram[:])
        w2 = const.tile([HID, C], F32, tag="w2")
        nc.sync.dma_start(w2[:], w2_dram[:])
        b2 = const.tile([C, 1], F32, tag="b2")
        nc.sync.dma_start(b2[:], b2_dram[:])
        wp = const.tile([C, (K + 1) * RANK], FP16, tag="wp")
        nc.sync.dma_start(wp[:], wp_dram[:])
        bp = const.tile([RANK, K + 1], F32, tag="bp")
        nc.sync.dma_start(bp[:], bp_dram[:])
        gam = const.tile([RANK, K + 1], FP16, tag="gam")
        nc.sync.dma_start(gam[:], gam_dram[:])
        dinvs = const.tile([3, NSH], F32, tag="dinvs")
        nc.sync.dma_start(dinvs[:], dinv_dram[:])
        sel3 = const.tile([3, 30], F32, tag="sel3")
        nc.sync.dma_start(sel3[:], sel3_dram[:])
        sel11 = const.tile([K + 1, (K + 1) * C], F32, tag="sel11")
        nc.sync.dma_start(sel11[:], sel11_dram[:])
        dB = [const.tile([C, NSH], F32, tag=f"dB{r}", name=f"dB{r}") for r in range(3)]
        with tc.tile_pool(name="psD", bufs=2, space="PSUM") as psD:
            for r in range(3):
                for s0, sw in STRIPS:
                    psd = psD.tile([C, 512], F32, space="PSUM", tag="psd",
                                   name=f"psd{r}")
                    nc.tensor.matmul(psd[:, 0:sw], sel3[:, r * 10:(r + 1) * 10],
                                     dinvs[:, s0:s0 + sw], start=True, stop=True)
                    nc.vector.tensor_copy(dB[r][:, s0:s0 + sw], psd[:, 0:sw])

        ident = const.tile([128, 128], F32, tag="ident")
        make_identity(nc, ident[:])

        u_stat = const.tile([128, GJT, UW], FP8, tag="u_stat")
        e_stat = const.tile([128, GJT, EW], FP16, tag="e_stat")

        # bf16 history of all Tx_k (for eta + hidden), f32 rotation state
        hist = [const.tile([C, NSH], FP16, tag=f"h{k}", name=f"hist{k}")
                for k in range(K + 1)]
        st = [const.tile([C, NSH], F32, tag=f"st{i}", name=f"state{i}")
              for i in range(3)]
        eT = const.tile([K + 1, NSH], F32, tag="eT")
        u_loc8 = const.tile([128, LT, UW], FP8, tag="u_loc8")
        hidT = const.tile([C, NSH], F32, tag="hidT")
        x1T = const.tile([HID, NSH], F32, tag="x1T")

        # ---------------- MLP ----------------
        KT = [(0, 128), (128, 128), (256, 128), (384, F_IN - 384)]
        with tc.tile_pool(name="psmlp", bufs=3, space="PSUM") as psmlp:
            fts = []
            for ki, (k0, kw) in enumerate(KT):
                ft = stream.tile([128, NSH], F32, tag="big", name=f"ft{ki}", bufs=3)
                nc.sync.dma_start(ft[0:kw, :], featT_dram[k0:k0 + kw, :])
                fts.append(ft)
            for s0, sw in STRIPS:
                ps = psmlp.tile([HID, 512], F32, space="PSUM", tag="psA", name="psA")
                for ki, (k0, kw) in enumerate(KT):
                    nc.tensor.matmul(
                        ps[:, 0:sw], w1[0:kw, ki, :], fts[ki][0:kw, s0:s0 + sw],
                        start=(ki == 0), stop=(ki == 3),
                    )
                nc.scalar.activation(x1T[:, s0:s0 + sw], ps[:, 0:sw], AF.Relu,
                                     bias=b1[:], scale=1.0)
            for s0, sw in STRIPS:
                ps2 = psmlp.tile([C, 512], F32, space="PSUM", tag="ps2", name="psB")
                nc.tensor.matmul(ps2[:, 0:sw], w2[:], x1T[:, s0:s0 + sw],
                                 start=True, stop=True)
                nc.scalar.activation(st[0][:, s0:s0 + sw], ps2[:, 0:sw], AF.Identity,
                                     bias=b2[:], scale=1.0)
        nc.vector.tensor_copy(hist[0][:], st[0][:])
        if DEBUG:
            nc.sync.dma_start(dump_tx[0], st[0][:])

        # ---------------- helpers ----------------
        def compute_eta(k):
            """e_k = tanh(Txk @ Wp[k] + bp[k]) @ (gamma[:,k]/3) into eT row k."""
            eRow = small.tile([1, NSH], F32, tag="eRow", name=f"eRow{k}")
            with tc.tile_pool(name=f"pse{k}", bufs=2, space="PSUM") as pse:
                for s0, sw in STRIPS:
                    psh = pse.tile([RANK, 512], F32, space="PSUM", tag="psh",
                                   name=f"psh{k}")
                    nc.tensor.matmul(psh[:, 0:sw], wp[:, k * RANK:(k + 1) * RANK],
                                     hist[k][:, s0:s0 + sw], start=True, stop=True)
                    hta = small.tile([RANK, 512], FP16, tag="hta", name=f"hta{k}")
                    nc.scalar.activation(hta[:, 0:sw], psh[:, 0:sw], AF.Tanh,
                                         bias=bp[:, k:k + 1], scale=1.0)
                    pse2 = pse.tile([1, 512], F32, space="PSUM", tag="pse2",
                                    name=f"pse2{k}")
                    nc.tensor.matmul(pse2[:, 0:sw], gam[:, k:k + 1], hta[:, 0:sw],
                                     start=True, stop=True)
                    nc.vector.tensor_copy(eRow[:, s0:s0 + sw], pse2[:, 0:sw])
            nc.sync.dma_start(eT[k:k + 1, :], eRow[:])

        def prep_u(cur, tag):
            """cur [C, NSH] f32 * dinv -> u_loc8 [128, LT, 20] fp8 hi/lo node-major."""
            uT = big.tile([C, NSH], F32, tag="uT", name=f"uT{tag}")
            nc.vector.tensor_tensor(out=uT[:], in0=cur[:],
                                    in1=dB[0][:], op=ALU.mult)
            with tc.tile_pool(name=f"psu{tag}", bufs=3, space="PSUM") as psu:
                for t in range(LT):
                    pw = 128 if t < LT - 1 else LLAST
                    psT = psu.tile([128, C], F32, space="PSUM", tag="psuT", name=f"psu{tag}_{t}")
                    nc.tensor.transpose(psT[0:pw, :], uT[:, t * 128:t * 128 + pw],
                                        ident[0:C, 0:C])
                    nc.vector.tensor_copy(u_loc8[0:pw, t, 0:10], psT[0:pw, :])
                    hif = small.tile([128, C], F32, tag="hif", name=f"hif{tag}_{t}")
                    nc.scalar.activation(hif[0:pw, :], u_loc8[0:pw, t, 0:10], AF.Copy)
                    r1 = small.tile([128, C], F32, tag="r1", name=f"r1{tag}_{t}")
                    nc.vector.tensor_tensor(out=r1[0:pw, :], in0=psT[0:pw, :],
                                            in1=hif[0:pw, :], op=ALU.subtract)
                    nc.scalar.activation(u_loc8[0:pw, t, 32:42], r1[0:pw, :],
                                         AF.Copy, scale=64.0)
                    midf = small.tile([128, C], F32, tag="midf", name=f"midf{tag}_{t}")
                    nc.scalar.activation(midf[0:pw, :], u_loc8[0:pw, t, 32:42],
                                         AF.Copy, scale=1.0 / 64.0)
                    r2 = small.tile([128, C], F32, tag="r2", name=f"r2{tag}_{t}")
                    nc.vector.tensor_tensor(out=r2[0:pw, :], in0=r1[0:pw, :],
                                            in1=midf[0:pw, :], op=ALU.subtract)
                    nc.scalar.activation(u_loc8[0:pw, t, 64:74], r2[0:pw, :],
                                         AF.Copy, scale=4096.0)

        def allgather_u():
            with tc.tile_critical():
                cc_sem = nc.alloc_semaphore(None)
                dma_sem = nc.alloc_semaphore(None)
                nc.sync.dma_start(out=ag_u_in[:], in_=u_loc8[:]).then_inc(dma_sem, 16)
                nc.sync.wait_ge(dma_sem, 16)
                nc.gpsimd.collective_compute(
                    "AllGather", ALU.bypass, replica_groups=RG,
                    ins=[ag_u_in[:]], outs=[ag_u_out[:]],
                ).then_inc(cc_sem, 1)
                nc.sync.wait_ge(cc_sem, 1)
                nc.sync.dma_start(
                    out=u_stat[:].rearrange("p (c t) x -> p c t x", c=NC),
                    in_=ag_u_out[:].rearrange("c p t x -> p c t x"),
                ).then_inc(dma_sem, 16)
                nc.sync.wait_ge(dma_sem, 32)

        # ---------------- Tx0 prep ----------------
        compute_eta(0)
        prep_u(st[0], "h0")

        # ---------------- hops ----------------
        cur_i, prev_i, free_i = 0, None, 1
        for k in range(1, K + 1):
            allgather_u()
            if DEBUG and k == 2:
                nc.gpsimd.dma_start(dump_ustat[:], u_stat[:])
            with tc.tile_pool(name=f"psh{k}", bufs=1, space="PSUM") as psh:
                pss = []
                for si, (s0, sw) in enumerate(STRIPS):
                    pss.append(psh.tile([74, 512], F32, space="PSUM", tag=f"s{si}",
                                        name=f"hop{k}s{si}"))
                for cg in range(NC):
                    r0 = cg * NSH
                    ach = stream.tile([128, LT, NSH], FP8, tag="big",
                                      name=f"ach{k}_{cg}")
                    nc.sync.dma_start(
                        ach[:, 0:LT - 1, :],
                        a_dram[r0:r0 + 128 * (LT - 1), :]
                        .rearrange("(t p) c -> p t c", p=128),
                    )
                    nc.sync.dma_start(ach[0:LLAST, LT - 1, :],
                                      a_dram[r0 + 128 * (LT - 1):r0 + NSH, :])
                    for t in range(LT):
                        kw = 128 if t < LT - 1 else LLAST
                        jg = cg * LT + t
                        for si, (s0, sw) in enumerate(STRIPS):
                            nc.tensor.matmul(
                                pss[si][:, 0:sw], u_stat[0:kw, jg, 0:74],
                                ach[0:kw, t, s0:s0 + sw],
                                start=(jg == 0), stop=(jg == GJT - 1),
                            )
                propT = big.tile([C, NSH], F32, tag="propT", name=f"propT{k}")
                for si, (s0, sw) in enumerate(STRIPS):
                    hiS = small.tile([C, 512], F32, tag="hiS", name=f"hiS{k}_{si}")
                    nc.vector.tensor_copy(hiS[:, 0:sw], pss[si][0:C, 0:sw])
                    miS = small.tile([C, 512], F32, tag="miS", name=f"miS{k}_{si}")
                    nc.scalar.activation(miS[:, 0:sw], pss[si][32:32 + C, 0:sw],
                                         AF.Copy, scale=1.0 / 64.0)
                    loS = small.tile([C, 512], F32, tag="loS", name=f"loS{k}_{si}")
                    nc.scalar.activation(loS[:, 0:sw], pss[si][64:64 + C, 0:sw],
                                         AF.Copy, scale=1.0 / 4096.0)
                    nc.vector.tensor_tensor(out=hiS[:, 0:sw],
                                            in0=hiS[:, 0:sw],
                                            in1=miS[:, 0:sw], op=ALU.add)
                    nc.vector.tensor_tensor(out=propT[:, s0:s0 + sw],
                                            in0=hiS[:, 0:sw],
                                            in1=loS[:, 0:sw], op=ALU.add)
            if DEBUG and k == 2:
                nc.sync.dma_start(dump_prop[:], propT[:])
            # chebyshev combine into a fresh state tile
            scale_rows = dB[1][:] if k == 1 else dB[2][:]
            scaled = big.tile([C, NSH], F32, tag="scaled", name=f"scaled{k}")
            nc.vector.tensor_tensor(out=scaled[:], in0=propT[:],
                                    in1=scale_rows[:], op=ALU.mult)
            nxt = st[free_i]
            if k == 1:
                nc.vector.tensor_copy(nxt[:], scaled[:])
            else:
                nc.vector.tensor_tensor(out=nxt[:], in0=scaled[:],
                                        in1=st[prev_i][:], op=ALU.subtract)
            nc.vector.tensor_copy(hist[k][:], nxt[:])
            if DEBUG:
                nc.sync.dma_start(dump_tx[k], nxt[:])
            prev_i, cur_i = cur_i, free_i
            free_i = 3 - cur_i - prev_i
            compute_eta(k)
            if k < K:
                prep_u(st[cur_i], f"h{k}")

        if DEBUG:
            nc.sync.dma_start(dump_e[:], eT[:])
        # ---------------- E allgather ----------------
        e_loc = const.tile([128, LT, EW], FP16, tag="e_loc")
        with tc.tile_pool(name="psE", bufs=3, space="PSUM") as psE:
            for t in range(LT):
                pw = 128 if t < LT - 1 else LLAST
                psT = psE.tile([128, K + 1], F32, space="PSUM", tag="psET", name=f"psE{t}")
                nc.tensor.transpose(psT[0:pw, :], eT[:, t * 128:t * 128 + pw],
                                    ident[0:K + 1, 0:K + 1])
                nc.vector.tensor_copy(e_loc[0:pw, t, 0:K + 1], psT[0:pw, :])
        with tc.tile_critical():
            cc_sem = nc.alloc_semaphore(None)
            dma_sem = nc.alloc_semaphore(None)
            nc.sync.dma_start(out=ag_e_in[:], in_=e_loc[:]).then_inc(dma_sem, 16)
            nc.sync.wait_ge(dma_sem, 16)
            nc.gpsimd.collective_compute(
                "AllGather", ALU.bypass, replica_groups=RG,
                ins=[ag_e_in[:]], outs=[ag_e_out[:]],
            ).then_inc(cc_sem, 1)
            nc.sync.wait_ge(cc_sem, 1)
            nc.sync.dma_start(
                out=e_stat[:].rearrange("p (c t) x -> p c t x", c=NC),
                in_=ag_e_out[:].rearrange("c p t x -> p c t x"),
            ).then_inc(dma_sem, 16)
            nc.sync.wait_ge(dma_sem, 32)

        # ---------------- CTC @ E + hidden ----------------
        with tc.tile_pool(name="psC", bufs=1, space="PSUM") as psC:
            pss = [psC.tile([K + 1, 512], F32, space="PSUM", tag=f"c{si}",
                            name=f"ctc{si}") for si in range(3)]
            for jg in range(GJT):
                cg, t = jg // LT, jg % LT
                kw = 128 if t < LT - 1 else LLAST
                row0 = cg * NSH + t * 128
                cj = stream.tile([128, NSH], FP16, tag="big", name=f"cj{jg}")
                nc.sync.dma_start(cj[0:kw, :], ctct_dram[row0:row0 + kw, :])
                for si, (s0, sw) in enumerate(STRIPS):
                    nc.tensor.matmul(
                        pss[si][:, 0:sw], e_stat[0:kw, jg, 0:K + 1],
                        cj[0:kw, s0:s0 + sw],
                        start=(jg == 0), stop=(jg == GJT - 1),
                    )
            # hidden = sum_k TxkT * (row k of Eta replicated to C partitions)
            etaS = big.tile([K + 1, NSH], F32, tag="etaS", name="etaS")
            for si, (s0, sw) in enumerate(STRIPS):
                nc.vector.tensor_copy(etaS[:, s0:s0 + sw], pss[si][:, 0:sw])
            if DEBUG:
                nc.sync.dma_start(dump_eta[:], etaS[:])
            with tc.tile_pool(name="psR", bufs=3, space="PSUM") as psR:
                for si, (s0, sw) in enumerate(STRIPS):
                    for k in range(K + 1):
                        psr = psR.tile([C, 512], F32, space="PSUM", tag="psr",
                                       name=f"psr{si}_{k}")
                        nc.tensor.matmul(psr[:, 0:sw], sel11[:, k * C:(k + 1) * C],
                                         etaS[:, s0:s0 + sw], start=True, stop=True)
                        tmp = small.tile([C, 512], F32, tag="htmp",
                                         name=f"htmp{si}_{k}")
                        nc.vector.tensor_tensor(
                            out=tmp[:, 0:sw], in0=hist[k][:, s0:s0 + sw],
                            in1=psr[:, 0:sw], op=ALU.mult)
                        if k == 0:
                            nc.vector.tensor_copy(hidT[:, s0:s0 + sw], tmp[:, 0:sw])
                        else:
                            nc.vector.tensor_tensor(out=hidT[:, s0:s0 + sw],
                                                    in0=hidT[:, s0:s0 + sw],
                                                    in1=tmp[:, 0:sw], op=ALU.add)

        if DEBUG:
            nc.sync.dma_start(dump_hid[:], hidT[:])
        # ---------------- log_softmax + out ----------------
        with tc.tile_pool(name="psS", bufs=3, space="PSUM") as psS:
            for t in range(LT):
                pw = 128 if t < LT - 1 else LLAST
                psT = psS.tile([128, C], F32, space="PSUM", tag="psST", name=f"psS{t}")
                nc.tensor.transpose(psT[0:pw, :], hidT[:, t * 128:t * 128 + pw],
                                    ident[0:C, 0:C])
                h = small.tile([128, C], F32, tag="hrow", name=f"hrow{t}")
                nc.vector.tensor_copy(h[0:pw, :], psT[0:pw, :])
                mx = small.tile([128, 1], F32, tag="mx", name=f"mx{t}")
                nc.vector.tensor_reduce(mx[0:pw, :], h[0:pw, :],
                                        axis=mybir.AxisListType.X, op=ALU.max)
                sh = small.tile([128, C], F32, tag="sh", name=f"sh{t}")
                nc.vector.tensor_scalar_sub(sh[0:pw, :], h[0:pw, :], mx[0:pw, :])
                ex = small.tile([128, C], F32, tag="ex", name=f"ex{t}")
                sm = small.tile([128, 1], F32, tag="sm", name=f"sm{t}")
                nc.scalar.activation(ex[0:pw, :], sh[0:pw, :], AF.Exp,
                                     accum_out=sm[0:pw, :])
                ls = small.tile([128, 1], F32, tag="ls", name=f"ls{t}")
                nc.scalar.activation(ls[0:pw, :], sm[0:pw, :], AF.Ln)
                o = small.tile([128, C], F32, tag="o", name=f"o{t}")
                nc.vector.tensor_scalar_sub(o[0:pw, :], sh[0:pw, :], ls[0:pw, :])
                nc.sync.dma_start(out_dram[t * 128:t * 128 + pw, :], o[0:pw, :])

    nc.compile()
    return nc


def _host_prep(feature, edges, CTC, W1, b1, W2, b2, gamma, Wp, bp):
    src = np.asarray(edges[0], dtype=np.int64)
    dst = np.asarray(edges[1], dtype=np.int64)
    nonself = src != dst
    s, d = src[nonself], dst[nonself]

    deg = np.bincount(s, minlength=N).astype(np.float64)
    dinv = np.where(deg > 0, 1.0 / np.sqrt(np.maximum(deg, 1e-30)), 0.0).astype(np.float32)

    counts = np.zeros((N, N), dtype=np.uint8)
    np.add.at(counts, (s, d), 1)
    lut = np.arange(256).astype(NP_FP8)
    a8 = lut[counts]          # [N, N] fp8, exact small ints
    # per-core-tiled row order: for core c, tiles of 128 (last 98); this is
    # just the identity permutation within each core range, concatenated - the
    # rows are already in that order, so no permutation needed. (Row blocks
    # are consecutive: core c rows [1250c, 1250c+1250).)

    feature = np.asarray(feature, dtype=np.float32)
    CTC = np.asarray(CTC, dtype=np.float32)

    sel3 = np.zeros((3, 30), dtype=np.float32)
    for r in range(3):
        sel3[r, r * 10:(r + 1) * 10] = 1.0
    sel11 = np.zeros((K + 1, (K + 1) * C), dtype=np.float32)
    for r in range(K + 1):
        sel11[r, r * C:(r + 1) * C] = 1.0

    in_maps = []
    for k in range(NC):
        r0, r1 = k * NSH, (k + 1) * NSH
        dloc = dinv[r0:r1]
        dinvs = np.stack([dloc, -dloc, -2.0 * dloc]).astype(np.float32)
        in_maps.append({
            "a8": np.ascontiguousarray(a8[:, r0:r1]),
            "featT": np.ascontiguousarray(feature[r0:r1].T),
            "ctct": np.ascontiguousarray(CTC[r0:r1].astype(np.float16).T),
            "w1": np.asarray(W1, dtype=np.float32),
            "b1": np.asarray(b1, dtype=np.float32).reshape(HID, 1),
            "w2": np.asarray(W2, dtype=np.float32),
            "b2": np.asarray(b2, dtype=np.float32).reshape(C, 1),
            "wp": np.ascontiguousarray(np.asarray(Wp, dtype=np.float32).transpose(1, 0, 2).reshape(C, (K + 1) * RANK)).astype(np.float16),
            "bp": np.ascontiguousarray(np.asarray(bp, dtype=np.float32).T),
            "gam": (np.asarray(gamma, dtype=np.float32) / RANK).astype(np.float16),
            "dinvs": dinvs,
            "sel3": sel3,
            "sel11": sel11,
        })
    return in_maps


def kernel(feature, edges, CTC, W1, b1, W2, b2, gamma, Wp, bp):
    from concourse.bass_utils import run_bass_kernel_spmd

    if "nc" not in _CACHE:
        _CACHE["nc"] = _build_program()
    nc = _CACHE["nc"]

    in_maps = _host_prep(feature, edges, CTC, W1, b1, W2, b2, gamma, Wp, bp)
    trace = bool(os.environ.get("GNN_TRACE"))
    res = run_bass_kernel_spmd(nc, in_maps, list(range(NC)), trace=trace)
    _CACHE["last_result"] = res
    out = np.concatenate([res.results[k]["out"] for k in range(NC)], axis=0)
    return out.astype(np.float32)



# revision 7
# speedup vs baseline: 1.3411x; 1.3411x over previous
"""CPFGNN Trainium2 kernel: 8-core SPMD Bass implementation (v2).

Math (exact simplifications of the reference):
  - lam = 2.0 always (w_off <= 0), so diag = 0 and prop(t) is a pure
    edge scatter-add: prop(t) = -D^-1/2 A^T D^-1/2 t, with A the
    (multi-)adjacency count matrix excluding self-loops and deg = out-degree.
  - The 11 CTC @ e_k matvecs batch into one CTC @ E (N x 11) pass.

Mapping (v2 changes vs v1):
  - A (fp8 exact counts) is RESIDENT in SBUF (loaded once, ~100KB/partition)
    instead of re-streamed every hop (saves ~112MB HBM traffic).
  - Hop matmuls use fp8 DoubleRow perf mode: 2 source k-tiles per
    instruction at 2x fp8 rate (the ragged 98-row tiles are zero-padded
    in both A and u so pairing is uniform).
  - MLP runs in bf16 (feature/W1/W2 cast on host).
  - Tx history is spilled to scratch DRAM per hop and streamed back in the
    final hidden combine (frees 25KB/partition of SBUF for A).
  - u allgather payload packed to 96 columns; identity shrunk to 32x32.
  - A dummy collective at program start absorbs the ~40us cold barrier
    under the A-load DMA + MLP.
"""
import os
import sys

sys.path.insert(0, "/opt/trn_rl_repo")

import numpy as np
import ml_dtypes
from contextlib import ExitStack

N = 10000
E_EDGES = 320000
F_IN = 500
HID = 64
C = 10
RANK = 3
K = 10
NC = 8
NSH = N // NC              # 1250 nodes per core
LT = (NSH + 127) // 128    # 10 local node tiles (last partial: 98)
LLAST = NSH - 128 * (LT - 1)  # 98
STRIPS = [(0, 512), (512, 512), (1024, NSH - 1024)]
# per-core row blocks: each core's 1250 nodes = 9 full 128-tiles + one 98-tile
GJT = NC * LT               # 80 global j-tiles in per-core-tiled order
NSHA = 1264                 # a_sb padded inner dim (%16==0 for DoubleRow)
UW = 80                     # packed fp8 u row (hi 0:10, mid 32:42, lo 64:74); %16==0 for DoubleRow ldweights
EW = 32                     # padded bf16 e row (11 used)

NP_FP8 = ml_dtypes.float8_e4m3
NP_BF16 = ml_dtypes.bfloat16

_CACHE = {}


def _build_program():
    import concourse.bass as bass
    import concourse.tile as tile
    from concourse import bacc, mybir
    from concourse.masks import make_identity

    dt = mybir.dt
    FP8 = dt.float8e4
    BF16 = dt.bfloat16
    FP16 = dt.float16
    F32 = dt.float32
    AF = mybir.ActivationFunctionType
    ALU = mybir.AluOpType
    DR = mybir.MatmulPerfMode.DoubleRow

    nc = bacc.Bacc("TRN2", target_bir_lowering=False, debug=False, num_devices=NC)

    # ---------------- DRAM I/O ----------------
    a_dram = nc.dram_tensor("a8", [N, NSH], FP8, kind="ExternalInput")
    featT_dram = nc.dram_tensor("featT", [F_IN, NSH], BF16, kind="ExternalInput")
    ctct_dram = nc.dram_tensor("ctct", [N, NSH], FP16, kind="ExternalInput")
    w1_dram = nc.dram_tensor("w1", [F_IN, HID], BF16, kind="ExternalInput")
    b1_dram = nc.dram_tensor("b1", [HID, 1], F32, kind="ExternalInput")
    w2_dram = nc.dram_tensor("w2", [HID, C], BF16, kind="ExternalInput")
    b2_dram = nc.dram_tensor("b2", [C, 1], F32, kind="ExternalInput")
    wp_dram = nc.dram_tensor("wp", [C, (K + 1) * RANK], FP16, kind="ExternalInput")
    bp_dram = nc.dram_tensor("bp", [RANK, K + 1], F32, kind="ExternalInput")
    gam_dram = nc.dram_tensor("gam", [RANK, K + 1], FP16, kind="ExternalInput")
    # rows: 0 = dinv_loc, 1 = -dinv_loc, 2 = -2*dinv_loc   (this core's range)
    dinv_dram = nc.dram_tensor("dinvs", [3, NSH], F32, kind="ExternalInput")
    sel3_dram = nc.dram_tensor("sel3", [3, 30], F32, kind="ExternalInput")
    sel11_dram = nc.dram_tensor("sel11", [K + 1, (K + 1) * C], F32, kind="ExternalInput")
    out_dram = nc.dram_tensor("out", [NSH, C], F32, kind="ExternalOutput")
    hist_dram = nc.dram_tensor("histd", [K + 1, C, NSH], FP16)
    DEBUG = bool(os.environ.get("GNN_DEBUG"))
    if DEBUG:
        dump_tx = nc.dram_tensor("dump_tx", [K + 1, C, NSH], F32, kind="ExternalOutput")
        dump_e = nc.dram_tensor("dump_e", [K + 1, NSH], F32, kind="ExternalOutput")
        dump_eta = nc.dram_tensor("dump_eta", [K + 1, NSH], F32, kind="ExternalOutput")
        dump_hid = nc.dram_tensor("dump_hid", [C, NSH], F32, kind="ExternalOutput")
        dump_ustat = nc.dram_tensor("dump_ustat", [128, GJT, UW], F32, kind="ExternalOutput")
        dump_prop = nc.dram_tensor("dump_prop", [C, NSH], F32, kind="ExternalOutput")

    warm_in = nc.dram_tensor("warm_in", [1, 16], FP8)
    warm_out = nc.dram_tensor("warm_out", [NC, 16], FP8, addr_space="Shared")
    ag_u_in = nc.dram_tensor("ag_u_in", [128, LT, UW], FP8)
    ag_u_out = nc.dram_tensor("ag_u_out", [NC, 128, LT, UW], FP8, addr_space="Shared")
    ag_e_in = nc.dram_tensor("ag_e_in", [128, LT, EW], FP16)
    ag_e_out = nc.dram_tensor("ag_e_out", [NC, 128, LT, EW], FP16, addr_space="Shared")

    RG = [list(range(NC))]

    with ExitStack() as ctx:
        tc = ctx.enter_context(tile.TileContext(nc))
        const = ctx.enter_context(tc.tile_pool(name="const", bufs=1))
        big = ctx.enter_context(tc.tile_pool(name="big", bufs=1))     # [C,NSH]-ish f32 temps
        small = ctx.enter_context(tc.tile_pool(name="small", bufs=3))  # small temps
        stream = ctx.enter_context(tc.tile_pool(name="stream", bufs=2))

        # ------------- warm-up collective: absorb the cold barrier -------------
        with tc.tile_critical():
            warm_sem = nc.alloc_semaphore(None)
            nc.gpsimd.collective_compute(
                "AllGather", ALU.bypass, replica_groups=RG,
                ins=[warm_in[:]], outs=[warm_out[:]],
            ).then_inc(warm_sem, 1)
            nc.sync.wait_ge(warm_sem, 1)

        # ------------- resident A (fp8 counts), loaded once -------------
        a_sb = const.tile([128, GJT, NSHA], FP8, tag="a_sb")
        for cg in range(NC):
            r0 = cg * NSH
            nc.vector.memset(a_sb[96:128, cg * LT + LT - 1, 0:NSH], 0)
            nc.sync.dma_start(
                a_sb[:, cg * LT:cg * LT + (LT - 1), 0:NSH],
                a_dram[r0:r0 + 128 * (LT - 1), :]
                .rearrange("(t p) c -> p t c", p=128),
            )
            nc.sync.dma_start(a_sb[0:LLAST, cg * LT + LT - 1, 0:NSH],
                              a_dram[r0 + 128 * (LT - 1):r0 + NSH, :])

        # ------------- resident constants -------------
        w1 = const.tile([128, 4, HID], BF16, tag="w1")
        nc.sync.dma_start(
            w1[:, 0:3, :], w1_dram[0:384, :].rearrange("(t p) c -> p t c", p=128)
        )
        nc.sync.dma_start(w1[0:F_IN - 384, 3, :], w1_dram[384:F_IN, :])
        b1 = const.tile([HID, 1], F32, tag="b1")
        nc.sync.dma_start(b1[:], b1_dram[:])
        w2 = const.tile([HID, C], BF16, tag="w2")
        nc.sync.dma_start(w2[:], w2_dram[:])
        b2 = const.tile([C, 1], F32, tag="b2")
        nc.sync.dma_start(b2[:], b2_dram[:])
        wp = const.tile([C, (K + 1) * RANK], FP16, tag="wp")
        nc.sync.dma_start(wp[:], wp_dram[:])
        bp = const.tile([RANK, K + 1], F32, tag="bp")
        nc.sync.dma_start(bp[:], bp_dram[:])
        gam = const.tile([RANK, K + 1], FP16, tag="gam")
        nc.sync.dma_start(gam[:], gam_dram[:])
        dinvs = const.tile([3, NSH], F32, tag="dinvs")
        nc.sync.dma_start(dinvs[:], dinv_dram[:])
        sel3 = const.tile([3, 30], F32, tag="sel3")
        nc.sync.dma_start(sel3[:], sel3_dram[:])
        sel11 = const.tile([K + 1, (K + 1) * C], F32, tag="sel11")
        nc.sync.dma_start(sel11[:], sel11_dram[:])
        dB = {r: const.tile([C, NSH], F32, tag=f"dB{r}", name=f"dB{r}") for r in (0, 2)}
        with tc.tile_pool(name="psD", bufs=2, space="PSUM") as psD:
            for r in (0, 2):
                for s0, sw in STRIPS:
                    psd = psD.tile([C, 512], F32, space="PSUM", tag="psd",
                                   name=f"psd{r}")
                    nc.tensor.matmul(psd[:, 0:sw], sel3[:, r * 10:(r + 1) * 10],
                                     dinvs[:, s0:s0 + sw], start=True, stop=True)
                    nc.vector.tensor_copy(dB[r][:, s0:s0 + sw], psd[:, 0:sw])

        ident = const.tile([32, 32], F32, tag="ident")
        make_identity(nc, ident[:])

        u_stat = const.tile([128, GJT, UW], FP8, tag="u_stat")
        e_stat = const.tile([128, GJT, EW], FP16, tag="e_stat")

        # fp16 current-Tx (for eta + hidden; full history spilled to DRAM)
        hist_cur = const.tile([C, NSH], FP16, tag="hist_cur")
        st = [const.tile([C, NSH], F32, tag=f"st{i}", name=f"state{i}")
              for i in range(3)]
        eT = const.tile([K + 1, NSH], F32, tag="eT")
        u_loc8 = const.tile([128, LT, UW], FP8, tag="u_loc8")
        nc.vector.memset(u_loc8[96:128, LT - 1, :], 0)
        hidT = const.tile([C, NSH], F32, tag="hidT")
        x1T = const.tile([HID, NSH], BF16, tag="x1T")

        # ---------------- MLP ----------------
        KT = [(0, 128), (128, 128), (256, 128), (384, F_IN - 384)]
        with tc.tile_pool(name="psmlp", bufs=3, space="PSUM") as psmlp:
            fts = []
            for ki, (k0, kw) in enumerate(KT):
                ft = stream.tile([128, NSH], BF16, tag="big", name=f"ft{ki}", bufs=4)
                nc.sync.dma_start(ft[0:kw, :], featT_dram[k0:k0 + kw, :])
                fts.append(ft)
            for s0, sw in STRIPS:
                ps = psmlp.tile([HID, 512], F32, space="PSUM", tag="psA", name="psA")
                for ki, (k0, kw) in enumerate(KT):
                    nc.tensor.matmul(
                        ps[:, 0:sw], w1[0:kw, ki, :], fts[ki][0:kw, s0:s0 + sw],
                        start=(ki == 0), stop=(ki == 3),
                    )
                nc.scalar.activation(x1T[:, s0:s0 + sw], ps[:, 0:sw], AF.Relu,
                                     bias=b1[:], scale=1.0)
            for s0, sw in STRIPS:
                ps2 = psmlp.tile([C, 512], F32, space="PSUM", tag="ps2", name="psB")
                nc.tensor.matmul(ps2[:, 0:sw], w2[:], x1T[:, s0:s0 + sw],
                                 start=True, stop=True)
                nc.scalar.activation(st[0][:, s0:s0 + sw], ps2[:, 0:sw], AF.Identity,
                                     bias=b2[:], scale=1.0)
        nc.vector.tensor_copy(hist_cur[:], st[0][:])
        nc.sync.dma_start(hist_dram[0], hist_cur[:])
        if DEBUG:
            nc.sync.dma_start(dump_tx[0], st[0][:])

        # ---------------- helpers ----------------
        def compute_eta(k):
            """e_k = tanh(Txk @ Wp[k] + bp[k]) @ (gamma[:,k]/3) into eT row k."""
            eRow = small.tile([1, NSH], F32, tag="eRow", name=f"eRow{k}", bufs=1)
            with tc.tile_pool(name=f"pse{k}", bufs=2, space="PSUM") as pse:
                for s0, sw in STRIPS:
                    psh = pse.tile([RANK, 512], F32, space="PSUM", tag="psh",
                                   name=f"psh{k}")
                    nc.tensor.matmul(psh[:, 0:sw], wp[:, k * RANK:(k + 1) * RANK],
                                     hist_cur[:, s0:s0 + sw], start=True, stop=True)
                    hta = small.tile([RANK, 512], FP16, tag="hta", name=f"hta{k}", bufs=2)
                    nc.scalar.activation(hta[:, 0:sw], psh[:, 0:sw], AF.Tanh,
                                         bias=bp[:, k:k + 1], scale=1.0)
                    pse2 = pse.tile([1, 512], F32, space="PSUM", tag="pse2",
                                    name=f"pse2{k}")
                    nc.tensor.matmul(pse2[:, 0:sw], gam[:, k:k + 1], hta[:, 0:sw],
                                     start=True, stop=True)
                    nc.vector.tensor_copy(eRow[:, s0:s0 + sw], pse2[:, 0:sw])
            nc.sync.dma_start(eT[k:k + 1, :], eRow[:])

        def prep_u(cur, tag):
            """cur [C, NSH] f32 * dinv -> u_loc8 [128, LT, UW] fp8 hi/mid/lo."""
            uT = big.tile([C, NSH], F32, tag="uT", name=f"uT{tag}")
            nc.vector.tensor_tensor(out=uT[:], in0=cur[:],
                                    in1=dB[0][:], op=ALU.mult)
            with tc.tile_pool(name=f"psu{tag}", bufs=3, space="PSUM") as psu:
                for t in range(LT):
                    pw = 128 if t < LT - 1 else LLAST
                    psT = psu.tile([128, C], F32, space="PSUM", tag="psuT", name=f"psu{tag}_{t}")
                    nc.tensor.transpose(psT[0:pw, :], uT[:, t * 128:t * 128 + pw],
                                        ident[0:C, 0:C])
                    nc.vector.tensor_copy(u_loc8[0:pw, t, 0:10], psT[0:pw, :])
                    hif = small.tile([128, C], F32, tag="hif", name=f"hif{tag}_{t}")
                    nc.scalar.activation(hif[0:pw, :], u_loc8[0:pw, t, 0:10], AF.Copy)
                    r1 = small.tile([128, C], F32, tag="r1", name=f"r1{tag}_{t}")
                    nc.vector.tensor_tensor(out=r1[0:pw, :], in0=psT[0:pw, :],
                                            in1=hif[0:pw, :], op=ALU.subtract)
                    nc.scalar.activation(u_loc8[0:pw, t, 32:42], r1[0:pw, :],
                                         AF.Copy, scale=64.0)
                    midf = small.tile([128, C], F32, tag="midf", name=f"midf{tag}_{t}")
                    nc.scalar.activation(midf[0:pw, :], u_loc8[0:pw, t, 32:42],
                                         AF.Copy, scale=1.0 / 64.0)
                    r2 = small.tile([128, C], F32, tag="r2", name=f"r2{tag}_{t}")
                    nc.vector.tensor_tensor(out=r2[0:pw, :], in0=r1[0:pw, :],
                                            in1=midf[0:pw, :], op=ALU.subtract)
                    nc.scalar.activation(u_loc8[0:pw, t, 64:74], r2[0:pw, :],
                                         AF.Copy, scale=4096.0)

        def allgather_u():
            with tc.tile_critical():
                cc_sem = nc.alloc_semaphore(None)
                dma_sem = nc.alloc_semaphore(None)
                nc.sync.dma_start(out=ag_u_in[:], in_=u_loc8[:]).then_inc(dma_sem, 16)
                nc.sync.wait_ge(dma_sem, 16)
                nc.gpsimd.collective_compute(
                    "AllGather", ALU.bypass, replica_groups=RG,
                    ins=[ag_u_in[:]], outs=[ag_u_out[:]],
                ).then_inc(cc_sem, 1)
                nc.sync.wait_ge(cc_sem, 1)
                nc.sync.dma_start(
                    out=u_stat[:].rearrange("p (c t) x -> p c t x", c=NC),
                    in_=ag_u_out[:].rearrange("c p t x -> p c t x"),
                ).then_inc(dma_sem, 16)
                nc.sync.wait_ge(dma_sem, 32)

        # ---------------- Tx0 prep ----------------
        compute_eta(0)
        prep_u(st[0], "h0")

        # ---------------- hops ----------------
        cur_i, prev_i, free_i = 0, None, 1
        for k in range(1, K + 1):
            allgather_u()
            if DEBUG and k == 2:
                nc.gpsimd.dma_start(dump_ustat[:], u_stat[:])
            with tc.tile_pool(name=f"psh{k}", bufs=1, space="PSUM") as psh:
                pss = []
                for si, (s0, sw) in enumerate(STRIPS):
                    pss.append(psh.tile([74, 512], F32, space="PSUM", tag=f"s{si}",
                                        name=f"hop{k}s{si}"))
                for jp in range(GJT // 2):
                    for si, (s0, sw) in enumerate(STRIPS):
                        nc.tensor.matmul(
                            pss[si][:, 0:sw],
                            u_stat[:, 2 * jp:2 * jp + 2, 0:74],
                            a_sb[:, 2 * jp:2 * jp + 2, s0:s0 + sw],
                            start=(jp == 0), stop=(jp == GJT // 2 - 1),
                            perf_mode=DR,
                        )
                propT = big.tile([C, NSH], F32, tag="propT", name=f"propT{k}")
                for si, (s0, sw) in enumerate(STRIPS):
                    hiS = small.tile([C, 512], F32, tag="hiS", name=f"hiS{k}_{si}", bufs=1)
                    nc.vector.tensor_copy(hiS[:, 0:sw], pss[si][0:C, 0:sw])
                    miS = small.tile([C, 512], F32, tag="miS", name=f"miS{k}_{si}", bufs=1)
                    nc.scalar.activation(miS[:, 0:sw], pss[si][32:32 + C, 0:sw],
                                         AF.Copy, scale=1.0 / 64.0)
                    loS = small.tile([C, 512], F32, tag="loS", name=f"loS{k}_{si}", bufs=1)
                    nc.scalar.activation(loS[:, 0:sw], pss[si][64:64 + C, 0:sw],
                                         AF.Copy, scale=1.0 / 4096.0)
                    nc.vector.tensor_tensor(out=hiS[:, 0:sw],
                                            in0=hiS[:, 0:sw],
                                            in1=miS[:, 0:sw], op=ALU.add)
                    nc.vector.tensor_tensor(out=propT[:, s0:s0 + sw],
                                            in0=hiS[:, 0:sw],
                                            in1=loS[:, 0:sw], op=ALU.add)
            if DEBUG and k == 2:
                nc.sync.dma_start(dump_prop[:], propT[:])
            # chebyshev combine into a fresh state tile (scale in-place)
            nc.vector.tensor_tensor(out=propT[:], in0=propT[:],
                                    in1=dB[2][:], op=ALU.mult)
            nxt = st[free_i]
            if k == 1:
                nc.scalar.activation(nxt[:], propT[:], AF.Copy, scale=0.5)
            else:
                nc.vector.tensor_tensor(out=nxt[:], in0=propT[:],
                                        in1=st[prev_i][:], op=ALU.subtract)
            nc.vector.tensor_copy(hist_cur[:], nxt[:])
            nc.sync.dma_start(hist_dram[k], hist_cur[:])
            if DEBUG:
                nc.sync.dma_start(dump_tx[k], nxt[:])
            prev_i, cur_i = cur_i, free_i
            free_i = 3 - cur_i - prev_i
            compute_eta(k)
            if k < K:
                prep_u(st[cur_i], f"h{k}")

        if DEBUG:
            nc.sync.dma_start(dump_e[:], eT[:])
        # ---------------- E allgather ----------------
        e_loc = const.tile([128, LT, EW], FP16, tag="e_loc")
        with tc.tile_pool(name="psE", bufs=3, space="PSUM") as psE:
            for t in range(LT):
                pw = 128 if t < LT - 1 else LLAST
                psT = psE.tile([128, K + 1], F32, space="PSUM", tag="psET", name=f"psE{t}")
                nc.tensor.transpose(psT[0:pw, :], eT[:, t * 128:t * 128 + pw],
                                    ident[0:K + 1, 0:K + 1])
                nc.vector.tensor_copy(e_loc[0:pw, t, 0:K + 1], psT[0:pw, :])
        with tc.tile_critical():
            cc_sem = nc.alloc_semaphore(None)
            dma_sem = nc.alloc_semaphore(None)
            nc.sync.dma_start(out=ag_e_in[:], in_=e_loc[:]).then_inc(dma_sem, 16)
            nc.sync.wait_ge(dma_sem, 16)
            nc.gpsimd.collective_compute(
                "AllGather", ALU.bypass, replica_groups=RG,
                ins=[ag_e_in[:]], outs=[ag_e_out[:]],
            ).then_inc(cc_sem, 1)
            nc.sync.wait_ge(cc_sem, 1)
            nc.sync.dma_start(
                out=e_stat[:].rearrange("p (c t) x -> p c t x", c=NC),
                in_=ag_e_out[:].rearrange("c p t x -> p c t x"),
            ).then_inc(dma_sem, 16)
            nc.sync.wait_ge(dma_sem, 32)

        # ---------------- CTC @ E + hidden ----------------
        with tc.tile_pool(name="psC", bufs=1, space="PSUM") as psC:
            pss = [psC.tile([K + 1, 512], F32, space="PSUM", tag=f"c{si}",
                            name=f"ctc{si}") for si in range(3)]
            for jg in range(GJT):
                cg, t = jg // LT, jg % LT
                kw = 128 if t < LT - 1 else LLAST
                row0 = cg * NSH + t * 128
                cj = stream.tile([128, NSH], FP16, tag="big", name=f"cj{jg}", bufs=4)
                nc.sync.dma_start(cj[0:kw, :], ctct_dram[row0:row0 + kw, :])
                for si, (s0, sw) in enumerate(STRIPS):
                    nc.tensor.matmul(
                        pss[si][:, 0:sw], e_stat[0:kw, jg, 0:K + 1],
                        cj[0:kw, s0:s0 + sw],
                        start=(jg == 0), stop=(jg == GJT - 1),
                    )
            # hidden = sum_k TxkT * (row k of Eta replicated to C partitions)
            etaS = big.tile([K + 1, NSH], F32, tag="etaS", name="etaS")
            for si, (s0, sw) in enumerate(STRIPS):
                nc.vector.tensor_copy(etaS[:, s0:s0 + sw], pss[si][:, 0:sw])
            if DEBUG:
                nc.sync.dma_start(dump_eta[:], etaS[:])
            with tc.tile_pool(name="psR", bufs=3, space="PSUM") as psR:
                for k in range(K + 1):
                    hr = stream.tile([C, NSH], FP16, tag="hrd", name=f"hr{k}", bufs=2)
                    nc.sync.dma_start(hr[:], hist_dram[k])
                    for si, (s0, sw) in enumerate(STRIPS):
                        psr = psR.tile([C, 512], F32, space="PSUM", tag="psr",
                                       name=f"psr{si}_{k}")
                        nc.tensor.matmul(psr[:, 0:sw], sel11[:, k * C:(k + 1) * C],
                                         etaS[:, s0:s0 + sw], start=True, stop=True)
                        tmp = small.tile([C, 512], F32, tag="htmp", bufs=1,
                                         name=f"htmp{si}_{k}")
                        nc.vector.tensor_tensor(
                            out=tmp[:, 0:sw], in0=hr[:, s0:s0 + sw],
                            in1=psr[:, 0:sw], op=ALU.mult)
                        if k == 0:
                            nc.vector.tensor_copy(hidT[:, s0:s0 + sw], tmp[:, 0:sw])
                        else:
                            nc.vector.tensor_tensor(out=hidT[:, s0:s0 + sw],
                                                    in0=hidT[:, s0:s0 + sw],
                                                    in1=tmp[:, 0:sw], op=ALU.add)

        if DEBUG:
            nc.sync.dma_start(dump_hid[:], hidT[:])
        # ---------------- log_softmax + out ----------------
        with tc.tile_pool(name="psS", bufs=3, space="PSUM") as psS:
            for t in range(LT):
                pw = 128 if t < LT - 1 else LLAST
                psT = psS.tile([128, C], F32, space="PSUM", tag="psST", name=f"psS{t}")
                nc.tensor.transpose(psT[0:pw, :], hidT[:, t * 128:t * 128 + pw],
                                    ident[0:C, 0:C])
                h = small.tile([128, C], F32, tag="hrow", name=f"hrow{t}")
                nc.vector.tensor_copy(h[0:pw, :], psT[0:pw, :])
                mx = small.tile([128, 1], F32, tag="mx", name=f"mx{t}")
                nc.vector.tensor_reduce(mx[0:pw, :], h[0:pw, :],
                                        axis=mybir.AxisListType.X, op=ALU.max)
                sh = small.tile([128, C], F32, tag="sh", name=f"sh{t}")
                nc.vector.tensor_scalar_sub(sh[0:pw, :], h[0:pw, :], mx[0:pw, :])
                ex = small.tile([128, C], F32, tag="ex", name=f"ex{t}")
                sm = small.tile([128, 1], F32, tag="sm", name=f"sm{t}")
                nc.scalar.activation(ex[0:pw, :], sh[0:pw, :], AF.Exp,
                                     accum_out=sm[0:pw, :])
                ls = small.tile([128, 1], F32, tag="ls", name=f"ls{t}")
                nc.scalar.activation(ls[0:pw, :], sm[0:pw, :], AF.Ln)
                o = small.tile([128, C], F32, tag="o", name=f"o{t}")
                nc.vector.tensor_scalar_sub(o[0:pw, :], sh[0:pw, :], ls[0:pw, :])
                nc.sync.dma_start(out_dram[t * 128:t * 128 + pw, :], o[0:pw, :])

    nc.compile()
    return nc


def _host_prep(feature, edges, CTC, W1, b1, W2, b2, gamma, Wp, bp):
    src = np.asarray(edges[0], dtype=np.int64)
    dst = np.asarray(edges[1], dtype=np.int64)
    nonself = src != dst
    s, d = src[nonself], dst[nonself]

    deg = np.bincount(s, minlength=N).astype(np.float64)
    dinv = np.where(deg > 0, 1.0 / np.sqrt(np.maximum(deg, 1e-30)), 0.0).astype(np.float32)

    counts = np.zeros((N, N), dtype=np.uint8)
    np.add.at(counts, (s, d), 1)
    lut = np.arange(256).astype(NP_FP8)
    a8 = lut[counts]          # [N, N] fp8, exact small ints

    feature = np.asarray(feature, dtype=np.float32)
    CTC = np.asarray(CTC, dtype=np.float32)

    sel3 = np.zeros((3, 30), dtype=np.float32)
    for r in range(3):
        sel3[r, r * 10:(r + 1) * 10] = 1.0
    sel11 = np.zeros((K + 1, (K + 1) * C), dtype=np.float32)
    for r in range(K + 1):
        sel11[r, r * C:(r + 1) * C] = 1.0

    in_maps = []
    for k in range(NC):
        r0, r1 = k * NSH, (k + 1) * NSH
        dloc = dinv[r0:r1]
        dinvs = np.stack([dloc, -dloc, -2.0 * dloc]).astype(np.float32)
        in_maps.append({
            "a8": np.ascontiguousarray(a8[:, r0:r1]),
            "featT": np.ascontiguousarray(feature[r0:r1].T).astype(NP_BF16),
            "ctct": np.ascontiguousarray(CTC[r0:r1].astype(np.float16).T),
            "w1": np.asarray(W1, dtype=np.float32).astype(NP_BF16),
            "b1": np.asarray(b1, dtype=np.float32).reshape(HID, 1),
            "w2": np.asarray(W2, dtype=np.float32).astype(NP_BF16),
            "b2": np.asarray(b2, dtype=np.float32).reshape(C, 1),
            "wp": np.ascontiguousarray(np.asarray(Wp, dtype=np.float32).transpose(1, 0, 2).reshape(C, (K + 1) * RANK)).astype(np.float16),
            "bp": np.ascontiguousarray(np.asarray(bp, dtype=np.float32).T),
            "gam": (np.asarray(gamma, dtype=np.float32) / RANK).astype(np.float16),
            "dinvs": dinvs,
            "sel3": sel3,
            "sel11": sel11,
        })
    return in_maps


def kernel(feature, edges, CTC, W1, b1, W2, b2, gamma, Wp, bp):
    from concourse.bass_utils import run_bass_kernel_spmd

    if "nc" not in _CACHE:
        _CACHE["nc"] = _build_program()
    nc = _CACHE["nc"]

    in_maps = _host_prep(feature, edges, CTC, W1, b1, W2, b2, gamma, Wp, bp)
    trace = bool(os.environ.get("GNN_TRACE"))
    res = run_bass_kernel_spmd(nc, in_maps, list(range(NC)), trace=trace)
    _CACHE["last_result"] = res
    out = np.concatenate([res.results[k]["out"] for k in range(NC)], axis=0)
    return out.astype(np.float32)


# revision 10
# speedup vs baseline: 1.4021x; 1.0455x over previous
"""CPFGNN Trainium2 kernel: 8-core SPMD Bass implementation (v2).

Math (exact simplifications of the reference):
  - lam = 2.0 always (w_off <= 0), so diag = 0 and prop(t) is a pure
    edge scatter-add: prop(t) = -D^-1/2 A^T D^-1/2 t, with A the
    (multi-)adjacency count matrix excluding self-loops and deg = out-degree.
  - The 11 CTC @ e_k matvecs batch into one CTC @ E (N x 11) pass.

Mapping (v2 changes vs v1):
  - A (fp8 exact counts) is RESIDENT in SBUF (loaded once, ~100KB/partition)
    instead of re-streamed every hop (saves ~112MB HBM traffic).
  - Hop matmuls use fp8 DoubleRow perf mode: 2 source k-tiles per
    instruction at 2x fp8 rate (the ragged 98-row tiles are zero-padded
    in both A and u so pairing is uniform).
  - MLP runs in bf16 (feature/W1/W2 cast on host).
  - Tx history is spilled to scratch DRAM per hop and streamed back in the
    final hidden combine (frees 25KB/partition of SBUF for A).
  - u allgather payload packed to 96 columns; identity shrunk to 32x32.
  - A dummy collective at program start absorbs the ~40us cold barrier
    under the A-load DMA + MLP.
"""
import os
import sys

sys.path.insert(0, "/opt/trn_rl_repo")

import numpy as np
import ml_dtypes
from contextlib import ExitStack

N = 10000
E_EDGES = 320000
F_IN = 500
HID = 64
C = 10
RANK = 3
K = 10
NC = 8
NSH = N // NC              # 1250 nodes per core
LT = (NSH + 127) // 128    # 10 local node tiles (last partial: 98)
LLAST = NSH - 128 * (LT - 1)  # 98
STRIPS = [(0, 512), (512, 512), (1024, NSH - 1024)]
# per-core row blocks: each core's 1250 nodes = 9 full 128-tiles + one 98-tile
GJT = NC * LT               # 80 global j-tiles in per-core-tiled order
NSHA = 1264                 # a_sb padded inner dim (%16==0 for DoubleRow)
UW = 80                     # packed fp8 u row (hi 0:10, mid 32:42, lo 64:74); %16==0 for DoubleRow ldweights
EW = 32                     # padded bf16 e row (11 used)

NP_FP8 = ml_dtypes.float8_e4m3
NP_BF16 = ml_dtypes.bfloat16

_CACHE = {}


def _build_program():
    import concourse.bass as bass
    import concourse.tile as tile
    from concourse import bacc, mybir
    from concourse.masks import make_identity

    dt = mybir.dt
    FP8 = dt.float8e4
    BF16 = dt.bfloat16
    FP16 = dt.float16
    F32 = dt.float32
    AF = mybir.ActivationFunctionType
    ALU = mybir.AluOpType
    DR = mybir.MatmulPerfMode.DoubleRow

    nc = bacc.Bacc("TRN2", target_bir_lowering=False, debug=False, num_devices=NC)

    # ---------------- DRAM I/O ----------------
    a_dram = nc.dram_tensor("a8", [N, NSH], FP8, kind="ExternalInput")
    featT_dram = nc.dram_tensor("featT", [F_IN, NSH], FP16, kind="ExternalInput")
    ctct_dram = nc.dram_tensor("ctct", [N, NSH], FP16, kind="ExternalInput")
    w1_dram = nc.dram_tensor("w1", [F_IN, HID], FP16, kind="ExternalInput")
    b1_dram = nc.dram_tensor("b1", [HID, 1], F32, kind="ExternalInput")
    w2_dram = nc.dram_tensor("w2", [HID, C], FP16, kind="ExternalInput")
    b2_dram = nc.dram_tensor("b2", [C, 1], F32, kind="ExternalInput")
    wp_dram = nc.dram_tensor("wp", [C, (K + 1) * RANK], FP16, kind="ExternalInput")
    bp_dram = nc.dram_tensor("bp", [RANK, K + 1], F32, kind="ExternalInput")
    gam_dram = nc.dram_tensor("gam", [RANK, K + 1], FP16, kind="ExternalInput")
    # dinv broadcast rows, built on host: db0 = dinv, db2 = -2*dinv
    db0_dram = nc.dram_tensor("db0", [C, NSH], F32, kind="ExternalInput")
    db2_dram = nc.dram_tensor("db2", [C, NSH], F32, kind="ExternalInput")
    sel11_dram = nc.dram_tensor("sel11", [K + 1, (K + 1) * C], FP16, kind="ExternalInput")
    out_dram = nc.dram_tensor("out", [NSH, C], F32, kind="ExternalOutput")
    hist_dram = nc.dram_tensor("histd", [K + 1, C, NSH], FP16)
    DEBUG = bool(os.environ.get("GNN_DEBUG"))
    if DEBUG:
        dump_tx = nc.dram_tensor("dump_tx", [K + 1, C, NSH], F32, kind="ExternalOutput")
        dump_e = nc.dram_tensor("dump_e", [K + 1, NSH], F32, kind="ExternalOutput")
        dump_eta = nc.dram_tensor("dump_eta", [K + 1, NSH], F32, kind="ExternalOutput")
        dump_hid = nc.dram_tensor("dump_hid", [C, NSH], F32, kind="ExternalOutput")
        dump_ustat = nc.dram_tensor("dump_ustat", [128, GJT, UW], F32, kind="ExternalOutput")
        dump_prop = nc.dram_tensor("dump_prop", [C, NSH], F32, kind="ExternalOutput")

    warm_in = nc.dram_tensor("warm_in", [1, 16], FP8)
    warm_out = nc.dram_tensor("warm_out", [NC, 16], FP8, addr_space="Shared")
    ag_u_in = nc.dram_tensor("ag_u_in", [128, LT, UW], FP8)
    ag_u_out = nc.dram_tensor("ag_u_out", [NC, 128, LT, UW], FP8, addr_space="Shared")
    ag_e_in = nc.dram_tensor("ag_e_in", [128, LT, EW], FP16)
    ag_e_out = nc.dram_tensor("ag_e_out", [NC, 128, LT, EW], FP16, addr_space="Shared")

    RG = [list(range(NC))]

    with ExitStack() as ctx:
        tc = ctx.enter_context(tile.TileContext(nc))
        const = ctx.enter_context(tc.tile_pool(name="const", bufs=1))
        big = ctx.enter_context(tc.tile_pool(name="big", bufs=1))     # [C,NSH]-ish f32 temps
        small = ctx.enter_context(tc.tile_pool(name="small", bufs=3))  # small temps
        stream = ctx.enter_context(tc.tile_pool(name="stream", bufs=2))

        # ------------- resident A (fp8 counts), loaded once -------------
        a_sb = const.tile([128, GJT, NSHA], FP8, tag="a_sb")
        for cg in range(NC):
            r0 = cg * NSH
            nc.vector.memset(a_sb[96:128, cg * LT + LT - 1, 0:NSH], 0)
            nc.sync.dma_start(
                a_sb[:, cg * LT:cg * LT + (LT - 1), 0:NSH],
                a_dram[r0:r0 + 128 * (LT - 1), :]
                .rearrange("(t p) c -> p t c", p=128),
            )
            nc.sync.dma_start(a_sb[0:LLAST, cg * LT + LT - 1, 0:NSH],
                              a_dram[r0 + 128 * (LT - 1):r0 + NSH, :])

        # ------------- resident constants -------------
        w1 = const.tile([128, 4, HID], FP16, tag="w1")
        nc.sync.dma_start(
            w1[:, 0:3, :], w1_dram[0:384, :].rearrange("(t p) c -> p t c", p=128)
        )
        nc.sync.dma_start(w1[0:F_IN - 384, 3, :], w1_dram[384:F_IN, :])
        b1 = const.tile([HID, 1], F32, tag="b1")
        nc.sync.dma_start(b1[:], b1_dram[:])
        w2 = const.tile([HID, C], FP16, tag="w2")
        nc.sync.dma_start(w2[:], w2_dram[:])
        b2 = const.tile([C, 1], F32, tag="b2")
        nc.sync.dma_start(b2[:], b2_dram[:])
        wp = const.tile([C, (K + 1) * RANK], FP16, tag="wp")
        nc.sync.dma_start(wp[:], wp_dram[:])
        bp = const.tile([RANK, K + 1], F32, tag="bp")
        nc.sync.dma_start(bp[:], bp_dram[:])
        gam = const.tile([RANK, K + 1], FP16, tag="gam")
        nc.sync.dma_start(gam[:], gam_dram[:])
        sel11 = const.tile([K + 1, (K + 1) * C], FP16, tag="sel11")
        nc.sync.dma_start(sel11[:], sel11_dram[:])
        dB = {r: const.tile([C, NSH], F32, tag=f"dB{r}", name=f"dB{r}") for r in (0, 2)}
        nc.sync.dma_start(dB[0][:], db0_dram[:])
        nc.sync.dma_start(dB[2][:], db2_dram[:])

        # warm-up collective: absorb the cold CC barrier under the loads
        with tc.tile_critical():
            warm_sem = nc.alloc_semaphore(None)
            nc.gpsimd.collective_compute(
                "AllGather", ALU.bypass, replica_groups=RG,
                ins=[warm_in[:]], outs=[warm_out[:]],
            ).then_inc(warm_sem, 1)
            nc.sync.wait_ge(warm_sem, 1)

        ident = const.tile([32, 32], F32, tag="ident")
        make_identity(nc, ident[:])

        u_stat = const.tile([128, GJT, UW], FP8, tag="u_stat")
        e_stat = const.tile([128, GJT, EW], FP16, tag="e_stat")

        # fp16 current-Tx (for eta + hidden; full history spilled to DRAM)
        hist_cur = const.tile([C, NSH], FP16, tag="hist_cur")
        st = [const.tile([C, NSH], F32, tag=f"st{i}", name=f"state{i}")
              for i in range(3)]
        eT = const.tile([K + 1, NSH], F32, tag="eT")
        u_loc8 = const.tile([128, LT, UW], FP8, tag="u_loc8")
        nc.vector.memset(u_loc8[96:128, LT - 1, :], 0)
        hidT = const.tile([C, NSH], F32, tag="hidT")
        x1T = const.tile([HID, NSH], FP16, tag="x1T")

        # ---------------- MLP ----------------
        KT = [(0, 128), (128, 128), (256, 128), (384, F_IN - 384)]
        with tc.tile_pool(name="psmlp", bufs=3, space="PSUM") as psmlp:
            fts = []
            for ki, (k0, kw) in enumerate(KT):
                ft = stream.tile([128, NSH], FP16, tag="big", name=f"ft{ki}", bufs=4)
                nc.sync.dma_start(ft[0:kw, :], featT_dram[k0:k0 + kw, :])
                fts.append(ft)
            for s0, sw in STRIPS:
                ps = psmlp.tile([HID, 512], F32, space="PSUM", tag="psA", name="psA")
                for ki, (k0, kw) in enumerate(KT):
                    nc.tensor.matmul(
                        ps[:, 0:sw], w1[0:kw, ki, :], fts[ki][0:kw, s0:s0 + sw],
                        start=(ki == 0), stop=(ki == 3),
                    )
                nc.scalar.activation(x1T[:, s0:s0 + sw], ps[:, 0:sw], AF.Relu,
                                     bias=b1[:], scale=1.0)
            for s0, sw in STRIPS:
                ps2 = psmlp.tile([C, 512], F32, space="PSUM", tag="ps2", name="psB")
                nc.tensor.matmul(ps2[:, 0:sw], w2[:], x1T[:, s0:s0 + sw],
                                 start=True, stop=True)
                nc.scalar.activation(st[0][:, s0:s0 + sw], ps2[:, 0:sw], AF.Identity,
                                     bias=b2[:], scale=1.0)
        nc.vector.tensor_copy(hist_cur[:], st[0][:])
        nc.sync.dma_start(hist_dram[0], hist_cur[:])
        if DEBUG:
            nc.sync.dma_start(dump_tx[0], st[0][:])

        # ---------------- helpers ----------------
        def compute_eta(k):
            """e_k = tanh(Txk @ Wp[k] + bp[k]) @ (gamma[:,k]/3) into eT row k."""
            eRow = small.tile([1, NSH], F32, tag="eRow", name=f"eRow{k}", bufs=1)
            with tc.tile_pool(name=f"pse{k}", bufs=2, space="PSUM") as pse:
                for s0, sw in STRIPS:
                    psh = pse.tile([RANK, 512], F32, space="PSUM", tag="psh",
                                   name=f"psh{k}")
                    nc.tensor.matmul(psh[:, 0:sw], wp[:, k * RANK:(k + 1) * RANK],
                                     hist_cur[:, s0:s0 + sw], start=True, stop=True)
                    hta = small.tile([RANK, 512], FP16, tag="hta", name=f"hta{k}", bufs=2)
                    nc.scalar.activation(hta[:, 0:sw], psh[:, 0:sw], AF.Tanh,
                                         bias=bp[:, k:k + 1], scale=1.0)
                    pse2 = pse.tile([1, 512], F32, space="PSUM", tag="pse2",
                                    name=f"pse2{k}")
                    nc.tensor.matmul(pse2[:, 0:sw], gam[:, k:k + 1], hta[:, 0:sw],
                                     start=True, stop=True)
                    nc.vector.tensor_copy(eRow[:, s0:s0 + sw], pse2[:, 0:sw])
            nc.sync.dma_start(eT[k:k + 1, :], eRow[:])

        def prep_u(cur, tag):
            """cur [C, NSH] f32 * dinv -> u_loc8 [128, LT, UW] fp8 hi/mid/lo."""
            uT = big.tile([C, NSH], F32, tag="uT", name=f"uT{tag}")
            nc.vector.tensor_tensor(out=uT[:], in0=cur[:],
                                    in1=dB[0][:], op=ALU.mult)
            with tc.tile_pool(name=f"psu{tag}", bufs=3, space="PSUM") as psu:
                for t in range(LT):
                    pw = 128 if t < LT - 1 else LLAST
                    psT = psu.tile([128, C], F32, space="PSUM", tag="psuT", name=f"psu{tag}_{t}")
                    nc.tensor.transpose(psT[0:pw, :], uT[:, t * 128:t * 128 + pw],
                                        ident[0:C, 0:C])
                    nc.vector.tensor_copy(u_loc8[0:pw, t, 0:10], psT[0:pw, :])
                    hif = small.tile([128, C], F32, tag="hif", name=f"hif{tag}_{t}")
                    nc.scalar.activation(hif[0:pw, :], u_loc8[0:pw, t, 0:10], AF.Copy)
                    r1 = small.tile([128, C], F32, tag="r1", name=f"r1{tag}_{t}")
                    nc.vector.tensor_tensor(out=r1[0:pw, :], in0=psT[0:pw, :],
                                            in1=hif[0:pw, :], op=ALU.subtract)
                    nc.scalar.activation(u_loc8[0:pw, t, 32:42], r1[0:pw, :],
                                         AF.Copy, scale=64.0)
                    midf = small.tile([128, C], F32, tag="midf", name=f"midf{tag}_{t}")
                    nc.scalar.activation(midf[0:pw, :], u_loc8[0:pw, t, 32:42],
                                         AF.Copy, scale=1.0 / 64.0)
                    r2 = small.tile([128, C], F32, tag="r2", name=f"r2{tag}_{t}")
                    nc.vector.tensor_tensor(out=r2[0:pw, :], in0=r1[0:pw, :],
                                            in1=midf[0:pw, :], op=ALU.subtract)
                    nc.scalar.activation(u_loc8[0:pw, t, 64:74], r2[0:pw, :],
                                         AF.Copy, scale=4096.0)

        def allgather_u():
            with tc.tile_critical():
                cc_sem = nc.alloc_semaphore(None)
                dma_sem = nc.alloc_semaphore(None)
                nc.sync.dma_start(out=ag_u_in[:], in_=u_loc8[:]).then_inc(dma_sem, 16)
                nc.sync.wait_ge(dma_sem, 16)
                nc.gpsimd.collective_compute(
                    "AllGather", ALU.bypass, replica_groups=RG,
                    ins=[ag_u_in[:]], outs=[ag_u_out[:]],
                ).then_inc(cc_sem, 1)
                nc.sync.wait_ge(cc_sem, 1)
                nc.sync.dma_start(
                    out=u_stat[:].rearrange("p (c t) x -> p c t x", c=NC),
                    in_=ag_u_out[:].rearrange("c p t x -> p c t x"),
                ).then_inc(dma_sem, 16)
                nc.sync.wait_ge(dma_sem, 32)

        # ---------------- Tx0 prep ----------------
        compute_eta(0)
        prep_u(st[0], "h0")

        # ---------------- hops ----------------
        cur_i, prev_i, free_i = 0, None, 1
        for k in range(1, K + 1):
            allgather_u()
            if DEBUG and k == 2:
                nc.gpsimd.dma_start(dump_ustat[:], u_stat[:])
            with tc.tile_pool(name=f"psh{k}", bufs=1, space="PSUM") as psh:
                pss = []
                for si, (s0, sw) in enumerate(STRIPS):
                    pss.append(psh.tile([74, 512], F32, space="PSUM", tag=f"s{si}",
                                        name=f"hop{k}s{si}"))
                for jp in range(GJT // 2):
                    for si, (s0, sw) in enumerate(STRIPS):
                        nc.tensor.matmul(
                            pss[si][:, 0:sw],
                            u_stat[:, 2 * jp:2 * jp + 2, 0:74],
                            a_sb[:, 2 * jp:2 * jp + 2, s0:s0 + sw],
                            start=(jp == 0), stop=(jp == GJT // 2 - 1),
                            perf_mode=DR,
                        )
                propT = big.tile([C, NSH], F32, tag="propT", name=f"propT{k}")
                for si, (s0, sw) in enumerate(STRIPS):
                    hiS = small.tile([C, 512], F32, tag="hiS", name=f"hiS{k}_{si}", bufs=1)
                    nc.vector.tensor_copy(hiS[:, 0:sw], pss[si][0:C, 0:sw])
                    miS = small.tile([C, 512], F32, tag="miS", name=f"miS{k}_{si}", bufs=1)
                    nc.scalar.activation(miS[:, 0:sw], pss[si][32:32 + C, 0:sw],
                                         AF.Copy, scale=1.0 / 64.0)
                    loS = small.tile([C, 512], F32, tag="loS", name=f"loS{k}_{si}", bufs=1)
                    nc.scalar.activation(loS[:, 0:sw], pss[si][64:64 + C, 0:sw],
                                         AF.Copy, scale=1.0 / 4096.0)
                    nc.vector.tensor_tensor(out=hiS[:, 0:sw],
                                            in0=hiS[:, 0:sw],
                                            in1=miS[:, 0:sw], op=ALU.add)
                    nc.vector.tensor_tensor(out=propT[:, s0:s0 + sw],
                                            in0=hiS[:, 0:sw],
                                            in1=loS[:, 0:sw], op=ALU.add)
            if DEBUG and k == 2:
                nc.sync.dma_start(dump_prop[:], propT[:])
            # chebyshev combine into a fresh state tile (scale in-place)
            nc.vector.tensor_tensor(out=propT[:], in0=propT[:],
                                    in1=dB[2][:], op=ALU.mult)
            nxt = st[free_i]
            if k == 1:
                nc.scalar.activation(nxt[:], propT[:], AF.Copy, scale=0.5)
            else:
                nc.vector.tensor_tensor(out=nxt[:], in0=propT[:],
                                        in1=st[prev_i][:], op=ALU.subtract)
            if DEBUG:
                nc.sync.dma_start(dump_tx[k], nxt[:])
            prev_i, cur_i = cur_i, free_i
            free_i = 3 - cur_i - prev_i
            if k < K:
                prep_u(st[cur_i], f"h{k}")
            nc.vector.tensor_copy(hist_cur[:], nxt[:])
            nc.sync.dma_start(hist_dram[k], hist_cur[:])
            compute_eta(k)

        if DEBUG:
            nc.sync.dma_start(dump_e[:], eT[:])
        # ---------------- E allgather ----------------
        e_loc = const.tile([128, LT, EW], FP16, tag="e_loc")
        with tc.tile_pool(name="psE", bufs=3, space="PSUM") as psE:
            for t in range(LT):
                pw = 128 if t < LT - 1 else LLAST
                psT = psE.tile([128, K + 1], F32, space="PSUM", tag="psET", name=f"psE{t}")
                nc.tensor.transpose(psT[0:pw, :], eT[:, t * 128:t * 128 + pw],
                                    ident[0:K + 1, 0:K + 1])
                nc.vector.tensor_copy(e_loc[0:pw, t, 0:K + 1], psT[0:pw, :])
        with tc.tile_critical():
            cc_sem = nc.alloc_semaphore(None)
            dma_sem = nc.alloc_semaphore(None)
            nc.sync.dma_start(out=ag_e_in[:], in_=e_loc[:]).then_inc(dma_sem, 16)
            nc.sync.wait_ge(dma_sem, 16)
            nc.gpsimd.collective_compute(
                "AllGather", ALU.bypass, replica_groups=RG,
                ins=[ag_e_in[:]], outs=[ag_e_out[:]],
            ).then_inc(cc_sem, 1)
            nc.sync.wait_ge(cc_sem, 1)
            nc.sync.dma_start(
                out=e_stat[:].rearrange("p (c t) x -> p c t x", c=NC),
                in_=ag_e_out[:].rearrange("c p t x -> p c t x"),
            ).then_inc(dma_sem, 16)
            nc.sync.wait_ge(dma_sem, 32)

        # ---------------- CTC @ E + hidden ----------------
        with tc.tile_pool(name="psC", bufs=1, space="PSUM") as psC:
            pss = [psC.tile([K + 1, 512], F32, space="PSUM", tag=f"c{si}",
                            name=f"ctc{si}") for si in range(3)]
            for jg in range(GJT):
                cg, t = jg // LT, jg % LT
                kw = 128 if t < LT - 1 else LLAST
                row0 = cg * NSH + t * 128
                cj = stream.tile([128, NSH], FP16, tag="big", name=f"cj{jg}", bufs=4)
                nc.sync.dma_start(cj[0:kw, :], ctct_dram[row0:row0 + kw, :])
                for si, (s0, sw) in enumerate(STRIPS):
                    nc.tensor.matmul(
                        pss[si][:, 0:sw], e_stat[0:kw, jg, 0:K + 1],
                        cj[0:kw, s0:s0 + sw],
                        start=(jg == 0), stop=(jg == GJT - 1),
                    )
            # hidden = sum_k TxkT * (row k of Eta replicated to C partitions)
            etaS = big.tile([K + 1, NSH], FP16, tag="etaS", name="etaS")
            for si, (s0, sw) in enumerate(STRIPS):
                nc.vector.tensor_copy(etaS[:, s0:s0 + sw], pss[si][:, 0:sw])
            if DEBUG:
                nc.sync.dma_start(dump_eta[:], etaS[:])
            with tc.tile_pool(name="psR", bufs=3, space="PSUM") as psR:
                for k in range(K + 1):
                    hr = stream.tile([C, NSH], FP16, tag="hrd", name=f"hr{k}", bufs=2)
                    nc.sync.dma_start(hr[:], hist_dram[k])
                    for si, (s0, sw) in enumerate(STRIPS):
                        psr = psR.tile([C, 512], F32, space="PSUM", tag="psr",
                                       name=f"psr{si}_{k}")
                        nc.tensor.matmul(psr[:, 0:sw], sel11[:, k * C:(k + 1) * C],
                                         etaS[:, s0:s0 + sw], start=True, stop=True)
                        tmp = small.tile([C, 512], F32, tag="htmp", bufs=1,
                                         name=f"htmp{si}_{k}")
                        nc.vector.tensor_tensor(
                            out=tmp[:, 0:sw], in0=hr[:, s0:s0 + sw],
                            in1=psr[:, 0:sw], op=ALU.mult)
                        if k == 0:
                            nc.vector.tensor_copy(hidT[:, s0:s0 + sw], tmp[:, 0:sw])
                        else:
                            nc.vector.tensor_tensor(out=hidT[:, s0:s0 + sw],
                                                    in0=hidT[:, s0:s0 + sw],
                                                    in1=tmp[:, 0:sw], op=ALU.add)

        if DEBUG:
            nc.sync.dma_start(dump_hid[:], hidT[:])
        # ---------------- log_softmax + out ----------------
        with tc.tile_pool(name="psS", bufs=3, space="PSUM") as psS:
            for t in range(LT):
                pw = 128 if t < LT - 1 else LLAST
                psT = psS.tile([128, C], F32, space="PSUM", tag="psST", name=f"psS{t}")
                nc.tensor.transpose(psT[0:pw, :], hidT[:, t * 128:t * 128 + pw],
                                    ident[0:C, 0:C])
                h = small.tile([128, C], F32, tag="hrow", name=f"hrow{t}")
                nc.vector.tensor_copy(h[0:pw, :], psT[0:pw, :])
                mx = small.tile([128, 1], F32, tag="mx", name=f"mx{t}")
                nc.vector.tensor_reduce(mx[0:pw, :], h[0:pw, :],
                                        axis=mybir.AxisListType.X, op=ALU.max)
                sh = small.tile([128, C], F32, tag="sh", name=f"sh{t}")
                nc.vector.tensor_scalar_sub(sh[0:pw, :], h[0:pw, :], mx[0:pw, :])
                ex = small.tile([128, C], F32, tag="ex", name=f"ex{t}")
                sm = small.tile([128, 1], F32, tag="sm", name=f"sm{t}")
                nc.scalar.activation(ex[0:pw, :], sh[0:pw, :], AF.Exp,
                                     accum_out=sm[0:pw, :])
                ls = small.tile([128, 1], F32, tag="ls", name=f"ls{t}")
                nc.scalar.activation(ls[0:pw, :], sm[0:pw, :], AF.Ln)
                o = small.tile([128, C], F32, tag="o", name=f"o{t}")
                nc.vector.tensor_scalar_sub(o[0:pw, :], sh[0:pw, :], ls[0:pw, :])
                nc.sync.dma_start(out_dram[t * 128:t * 128 + pw, :], o[0:pw, :])

    nc.compile()
    return nc


def _host_prep(feature, edges, CTC, W1, b1, W2, b2, gamma, Wp, bp):
    src = np.asarray(edges[0], dtype=np.int64)
    dst = np.asarray(edges[1], dtype=np.int64)
    nonself = src != dst
    s, d = src[nonself], dst[nonself]

    deg = np.bincount(s, minlength=N).astype(np.float64)
    dinv = np.where(deg > 0, 1.0 / np.sqrt(np.maximum(deg, 1e-30)), 0.0).astype(np.float32)

    counts = np.zeros((N, N), dtype=np.uint8)
    np.add.at(counts, (s, d), 1)
    lut = np.arange(256).astype(NP_FP8)
    a8 = lut[counts]          # [N, N] fp8, exact small ints

    feature = np.asarray(feature, dtype=np.float32)
    CTC = np.asarray(CTC, dtype=np.float32)

    sel11 = np.zeros((K + 1, (K + 1) * C), dtype=np.float32)
    for r in range(K + 1):
        sel11[r, r * C:(r + 1) * C] = 1.0

    in_maps = []
    for k in range(NC):
        r0, r1 = k * NSH, (k + 1) * NSH
        dloc = dinv[r0:r1]
        db0 = np.ascontiguousarray(np.broadcast_to(dloc, (C, NSH))).astype(np.float32)
        db2 = np.ascontiguousarray(np.broadcast_to(-2.0 * dloc, (C, NSH))).astype(np.float32)
        in_maps.append({
            "a8": np.ascontiguousarray(a8[:, r0:r1]),
            "featT": np.ascontiguousarray(feature[r0:r1].T).astype(np.float16),
            "ctct": np.ascontiguousarray(CTC[r0:r1].astype(np.float16).T),
            "w1": np.asarray(W1, dtype=np.float32).astype(np.float16),
            "b1": np.asarray(b1, dtype=np.float32).reshape(HID, 1),
            "w2": np.asarray(W2, dtype=np.float32).astype(np.float16),
            "b2": np.asarray(b2, dtype=np.float32).reshape(C, 1),
            "wp": np.ascontiguousarray(np.asarray(Wp, dtype=np.float32).transpose(1, 0, 2).reshape(C, (K + 1) * RANK)).astype(np.float16),
            "bp": np.ascontiguousarray(np.asarray(bp, dtype=np.float32).T),
            "gam": (np.asarray(gamma, dtype=np.float32) / RANK).astype(np.float16),
            "db0": db0,
            "db2": db2,
            "sel11": sel11.astype(np.float16),
        })
    return in_maps


def kernel(feature, edges, CTC, W1, b1, W2, b2, gamma, Wp, bp):
    from concourse.bass_utils import run_bass_kernel_spmd

    if "nc" not in _CACHE:
        _CACHE["nc"] = _build_program()
    nc = _CACHE["nc"]

    in_maps = _host_prep(feature, edges, CTC, W1, b1, W2, b2, gamma, Wp, bp)
    trace = bool(os.environ.get("GNN_TRACE"))
    res = run_bass_kernel_spmd(nc, in_maps, list(range(NC)), trace=trace)
    _CACHE["last_result"] = res
    out = np.concatenate([res.results[k]["out"] for k in range(NC)], axis=0)
    return out.astype(np.float32)
